# revision 1
# baseline (speedup 1.0000x reference)
"""Multi-head attention (4x2048x512, 8 heads of 64) on 8 Trainium2 NeuronCores.

Sharding: core c handles batch b = c//2 and head-group g = c%2 (4 heads each).
The host pre-transposes x[b] -> xT [512, 2048], slices the QKV / out
projection weights per core, and casts them to bf16.  Each core computes

    qT,kT  = w_qk.T @ xT          (per head, [64, 2048] each)
    v      = xT.T  @ w_v          (natural [2048, 256], +ones column)
    sT     = kT.T  @ qT           (scores transposed, [j, i], fp32 in PSUM)
    p      = exp(sT / 8)          (flash-style over j-chunks, bf16 out)
    oT     = v_aug.T @ p          (accumulated over j; row 64 = softmax denom)
    attT   = oT[0:64] / denom     (denominator kept in fp32)
    yT     = w_out_slice.T @ attT  ([512, 2048] fp32 partial)

and the host reduces: out[b] = (yT[2b] + yT[2b+1]).T + b_out.

All matmuls run in bf16 (fp32 PSUM accumulation).  float32r would be more
accurate but measures ~4-5us per matmul on this hardware (~25x the bf16
rate), so bf16 it is.  The softmax exp runs on the Scalar engine straight
out of PSUM with the 1/8 scale folded into the activation; the softmax
numerator and denominator are both sums of the same bf16-rounded p, so the
normalization cancels most of the rounding error.
"""

import numpy as np

N = 2048          # sequence length
DMODEL = 512      # model dim
DH = 64           # head dim
HEADS = 4         # heads per core
N_CORES = 8
I_HALF = N // 2   # flash loop processes i in halves of 1024
JC = N // 128     # 16 j-chunks per head
KO = DMODEL // 128  # 4 contraction chunks of the model dim

_CACHE = {}


def _fixup_drains(nc, mybir):
    """walrus in this container rejects instructions carrying multiple sem
    waits ("Too many sync wait commands", e.g. on Drain and on the fused
    LDWEIGHTS of Matmult); hoist all-but-one wait onto single-wait NoOps
    right before the instruction — semantically identical (the engine
    stalls at the NoOps instead)."""
    for fn in nc.m.functions:
        for blk in fn.blocks:
            new = []
            for inst in blk.instructions:
                si = getattr(inst, "sync_info", None)
                if si is not None and si.on_wait:
                    keep = 0 if isinstance(inst, mybir.InstDrain) else 1
                    waits = list(si.on_wait)
                    if len(waits) > keep:
                        extra, rest = waits[keep:], waits[:keep]
                        for j, w in enumerate(extra):
                            nop = mybir.InstNoOp(
                                name=f"{inst.name}-ws{j}", ins=[], outs=[]
                            )
                            nop.engine = inst.engine
                            nop.sync_info = mybir.SyncInfo(on_wait=[w], on_update=[])
                            new.append(nop)
                        si.on_wait = rest
                new.append(inst)
            blk.instructions = new


def build_nc(repeat=1, fixup=True, loop=False, stages=4):
    """Build the per-core Bass program (identical on all 8 cores)."""
    import contextlib

    import concourse.bass as bass
    import concourse.tile as tile
    from concourse import mybir

    f32 = mybir.dt.float32
    bf16 = mybir.dt.bfloat16

    nc = bass.Bass()
    xt = nc.dram_tensor("xt", [DMODEL, N], bf16, kind="ExternalInput")
    wqk = nc.dram_tensor("wqk", [DMODEL, HEADS * 128], bf16, kind="ExternalInput")
    wv = nc.dram_tensor("wv", [DMODEL, HEADS * DH], bf16, kind="ExternalInput")
    wo = nc.dram_tensor("wo", [HEADS * DH, DMODEL], bf16, kind="ExternalInput")
    yt = nc.dram_tensor("yt", [DMODEL, N], f32, kind="ExternalOutput")

    with tile.TileContext(nc) as tc:
        with (
            tc.tile_pool(name="singles", bufs=1) as singles,
        ):
            x_sb = singles.tile([128, KO, N], bf16)
            wqk_sb = singles.tile([128, KO, HEADS, 128], bf16)
            wv_sb = singles.tile([128, KO, HEADS * DH], bf16)
            wo_sb = singles.tile([128, 2, DMODEL], bf16)
            q_sb = singles.tile([DH, HEADS, N], bf16)
            k_sb = singles.tile([DH, HEADS, N], bf16)
            v_sb = singles.tile([128, JC, HEADS, 2 * DH], bf16)
            att_sb = singles.tile([128, 2, N], bf16)

            nc.sync.dma_start(x_sb[:], xt.ap().rearrange("(ko p) n -> p ko n", p=128))
            nc.sync.dma_start(
                wqk_sb[:], wqk.ap().rearrange("(ko p) (h m) -> p ko h m", p=128, m=128)
            )
            nc.sync.dma_start(wv_sb[:], wv.ap().rearrange("(ko p) v -> p ko v", p=128))
            nc.sync.dma_start(wo_sb[:], wo.ap().rearrange("(c p) n -> p c n", p=128))
            nc.vector.memset(v_sb[:, :, :, DH + 1 :], 0.0)
            nc.vector.memset(v_sb[:, :, :, DH : DH + 1], 1.0)
            f16 = mybir.dt.float16
            ones_col = singles.tile([1, DH], f16)
            nc.vector.memset(ones_col[:], 1.0)

            loop_cm = tc.For_i(0, repeat, 1) if loop else contextlib.nullcontext()
            with loop_cm:
              for rep in range(1 if loop else repeat):
                # ---- phase A: qkT per head + v (natural layout) ----
                with (
                    tc.tile_pool(name="ps_qk", bufs=1, space="PSUM") as ps_qk,
                    tc.tile_pool(name="ps_v", bufs=4, space="PSUM") as ps_v,
                ):
                    for h in range(HEADS):
                        pqk = ps_qk.tile([128, N], f32)
                        for t in range(N // 512):
                            for ko in range(KO):
                                nc.tensor.matmul(
                                    pqk[:, t * 512 : (t + 1) * 512],
                                    wqk_sb[:, ko, h, :],
                                    x_sb[:, ko, t * 512 : (t + 1) * 512],
                                    start=(ko == 0),
                                    stop=(ko == KO - 1),
                                )
                        nc.vector.tensor_copy(q_sb[:, h, :], pqk[0:DH, :])
                        nc.vector.tensor_copy(k_sb[:, h, :], pqk[DH:128, :])
                    for jc in range(JC):
                        pv = ps_v.tile([128, HEADS * DH], f32)
                        for ko in range(KO):
                            nc.tensor.matmul(
                                pv[:],
                                x_sb[:, ko, jc * 128 : (jc + 1) * 128],
                                wv_sb[:, ko, :],
                                start=(ko == 0),
                                stop=(ko == KO - 1),
                            )
                        nc.vector.tensor_copy(
                            v_sb[:, jc, :, 0:DH],
                            pv[:].rearrange("p (h d) -> p h d", d=DH),
                        )

                # ---- phase B: flash attention over (i_half, head, j-chunk) ----
                if stages < 2:
                    continue
                with (
                    tc.tile_pool(name="ps_s", bufs=2, space="PSUM") as ps_s,
                    tc.tile_pool(name="ps_o", bufs=2, space="PSUM") as ps_o,
                    tc.tile_pool(name="p_sb", bufs=3) as p_pool,
                    tc.tile_pool(name="den_row", bufs=2) as den_pool,
                    tc.tile_pool(name="rep_sb", bufs=2) as rep_pool,
                ):
                    for ih in range(2):
                        i0 = ih * I_HALF
                        for h in range(HEADS):
                            o = ps_o.tile([2 * DH, I_HALF], f32)
                            for jc in range(JC):
                                s = ps_s.tile([128, I_HALF], f32)
                                for t in range(I_HALF // 512):
                                    nc.tensor.matmul(
                                        s[:, t * 512 : (t + 1) * 512],
                                        k_sb[:, h, jc * 128 : (jc + 1) * 128],
                                        q_sb[:, h, i0 + t * 512 : i0 + (t + 1) * 512],
                                        start=True,
                                        stop=True,
                                    )
                                p = p_pool.tile([128, I_HALF], bf16)
                                nc.scalar.activation(
                                    p[:], s[:], mybir.ActivationFunctionType.Exp,
                                    scale=0.125,
                                )
                                for t in range(I_HALF // 512):
                                    nc.tensor.matmul(
                                        o[:, t * 512 : (t + 1) * 512],
                                        v_sb[:, jc, h, :],
                                        p[:, t * 512 : (t + 1) * 512],
                                        start=(jc == 0),
                                        stop=(jc == JC - 1),
                                    )
                            den_row = den_pool.tile([1, I_HALF], f32)
                            nc.vector.tensor_copy(den_row[:], o[DH : DH + 1, :])
                            rec_row = den_pool.tile([1, I_HALF], f16)
                            with nc.allow_low_precision(
                                reason="softmax denom reciprocal; fp16 has "
                                "10-bit mantissa, plenty for a scale factor"
                            ):
                                nc.vector.reciprocal(rec_row[:], den_row[:])
                            # broadcast rec_row across 64 partitions via a
                            # K=1 outer product on the PE (shares the scores
                            # pool's PSUM slots), then one multiply
                            rep_ps = ps_s.tile([DH, I_HALF], f32, tag="s")
                            for t in range(I_HALF // 512):
                                nc.tensor.matmul(
                                    rep_ps[:, t * 512 : (t + 1) * 512],
                                    ones_col[:],
                                    rec_row[:, t * 512 : (t + 1) * 512],
                                    start=True, stop=True,
                                )
                            rep = rep_pool.tile([DH, I_HALF], f32)
                            nc.vector.tensor_copy(rep[:], rep_ps[:])
                            nc.vector.tensor_mul(
                                att_sb[(h % 2) * DH : (h % 2 + 1) * DH, h // 2,
                                       i0 : i0 + I_HALF],
                                o[0:DH, :],
                                rep[:],
                            )

                # ---- phase C: output projection yT = wo.T @ attT ----
                if stages < 4:
                    continue
                with (
                    tc.tile_pool(name="ps_y", bufs=4, space="PSUM") as ps_y,
                    tc.tile_pool(name="y_sb", bufs=2) as y_pool,
                ):
                    for m in range(KO):
                        y_row = y_pool.tile([128, N], f32)
                        for t in range(N // 512):
                            py = ps_y.tile([128, 512], f32)
                            for c in range(2):
                                nc.tensor.matmul(
                                    py[:],
                                    wo_sb[:, c, m * 128 : (m + 1) * 128],
                                    att_sb[:, c, t * 512 : (t + 1) * 512],
                                    start=(c == 0),
                                    stop=(c == 1),
                                )
                            nc.vector.tensor_copy(y_row[:, t * 512 : (t + 1) * 512], py[:])
                        nc.sync.dma_start(
                            yt.ap().rearrange("(mo p) n -> p mo n", p=128)[:, m, :],
                            y_row[:],
                        )

    if fixup:
        _fixup_drains(nc, mybir)
    return nc


def _per_core_inputs(x, w_qkv, w_out):
    """Slice + transpose the full inputs into the 8 per-core input maps."""
    import ml_dtypes

    bf16 = ml_dtypes.bfloat16
    ins = []
    for c in range(N_CORES):
        b, g = c // 2, c % 2
        xt = np.ascontiguousarray(x[b].T).astype(bf16)          # [512, 2048]
        wq = w_qkv[:, g * 256 : (g + 1) * 256]                  # [512, 256]
        wk = w_qkv[:, 512 + g * 256 : 512 + (g + 1) * 256]
        wv = w_qkv[:, 1024 + g * 256 : 1024 + (g + 1) * 256]
        # per head: [w_q_h | w_k_h] -> [512, 4, 128]
        wqk = np.empty((DMODEL, HEADS, 128), np.float32)
        for h in range(HEADS):
            wqk[:, h, :DH] = wq[:, h * DH : (h + 1) * DH]
            wqk[:, h, DH:] = wk[:, h * DH : (h + 1) * DH]
        wo = w_out[g * 256 : (g + 1) * 256, :]                  # [256, 512]
        ins.append(
            {
                "xt": xt,
                "wqk": np.ascontiguousarray(wqk.reshape(DMODEL, HEADS * 128)).astype(bf16),
                "wv": np.ascontiguousarray(wv).astype(bf16),
                "wo": np.ascontiguousarray(wo).astype(bf16),
            }
        )
    return ins


def run_on_hw(x, w_qkv, w_out, b_out, repeat=1, loop=False):
    from concourse.bass_utils import run_bass_kernel_spmd

    key = ("nc", repeat, loop)
    if key not in _CACHE:
        _CACHE[key] = build_nc(repeat=repeat, loop=loop)
    nc = _CACHE[key]
    ins = _per_core_inputs(
        np.asarray(x, np.float32),
        np.asarray(w_qkv, np.float32),
        np.asarray(w_out, np.float32),
    )
    res = run_bass_kernel_spmd(nc, ins, core_ids=list(range(N_CORES)))
    yts = [res.results[c]["yt"] for c in range(N_CORES)]
    b_out = np.asarray(b_out, np.float32)
    out = np.stack(
        [(yts[2 * b] + yts[2 * b + 1]).T + b_out[None, :] for b in range(4)]
    )
    return out.astype(np.float32)


def kernel(x, w_qkv, w_out, b_out):
    return run_on_hw(x, w_qkv, w_out, b_out, repeat=1)



# revision 12
# speedup vs baseline: 493.4574x; 493.4574x over previous
"""Multi-head attention (4x2048x512, 8 heads of 64) on 8 Trainium2 NeuronCores.

Sharding: core c handles batch b = c//2 and head-group g = c%2 (4 heads each).
The host pre-transposes x[b] -> xT [512, 2048], slices the QKV / out
projection weights per core, and casts them to bf16.  Each core computes

    qT,kT  = w_qk.T @ xT          (per head, [64, 2048] each)
    v      = xT.T  @ w_v          (natural [2048, 256], +ones column)
    sT     = kT.T  @ qT           (scores transposed, [j, i], fp32 in PSUM)
    p      = exp(sT / 8)          (flash-style over j-chunks, bf16 out)
    oT     = v_aug.T @ p          (accumulated over j; row 64 = softmax denom)
    attT   = oT[0:64] / denom     (denominator kept in fp32)
    yT     = w_out_slice.T @ attT  ([512, 2048] fp32 partial)

and the host reduces: out[b] = (yT[2b] + yT[2b+1]).T + b_out.

All matmuls run in bf16 (fp32 PSUM accumulation).  float32r would be more
accurate but measures ~4-5us per matmul on this hardware (~25x the bf16
rate), so bf16 it is.  The softmax exp runs on the Scalar engine straight
out of PSUM with the 1/8 scale folded into the activation; the softmax
numerator and denominator are both sums of the same bf16-rounded p, so the
normalization cancels most of the rounding error.
"""

import numpy as np

N = 2048          # sequence length
DMODEL = 512      # model dim
DH = 64           # head dim
HEADS = 4         # heads per core
N_CORES = 8
I_HALF = N // 2   # flash loop processes i in halves of 1024
JC = N // 128     # 16 j-chunks per head
KO = DMODEL // 128  # 4 contraction chunks of the model dim

_CACHE = {}


def _fixup_drains(nc, mybir):
    """walrus in this container rejects instructions carrying multiple sem
    waits ("Too many sync wait commands", e.g. on Drain and on the fused
    LDWEIGHTS of Matmult); hoist all-but-one wait onto single-wait NoOps
    right before the instruction — semantically identical (the engine
    stalls at the NoOps instead)."""
    for fn in nc.m.functions:
        for blk in fn.blocks:
            new = []
            for inst in blk.instructions:
                si = getattr(inst, "sync_info", None)
                if si is not None and si.on_wait:
                    keep = 0 if isinstance(inst, mybir.InstDrain) else 1
                    waits = list(si.on_wait)
                    if len(waits) > keep:
                        extra, rest = waits[keep:], waits[:keep]
                        for j, w in enumerate(extra):
                            nop = mybir.InstNoOp(
                                name=f"{inst.name}-ws{j}", ins=[], outs=[]
                            )
                            nop.engine = inst.engine
                            nop.sync_info = mybir.SyncInfo(on_wait=[w], on_update=[])
                            new.append(nop)
                        si.on_wait = rest
                new.append(inst)
            blk.instructions = new


def build_nc(repeat=1, fixup=True, loop=False, stages=4):
    """Build the per-core Bass program (identical on all 8 cores)."""
    import contextlib

    import concourse.bass as bass
    import concourse.tile as tile
    from concourse import mybir

    f32 = mybir.dt.float32
    bf16 = mybir.dt.bfloat16

    nc = bass.Bass()
    xt = nc.dram_tensor("xt", [DMODEL, N], bf16, kind="ExternalInput")
    wqk = nc.dram_tensor("wqk", [DMODEL, HEADS * 128], bf16, kind="ExternalInput")
    wv = nc.dram_tensor("wv", [DMODEL, HEADS * DH], bf16, kind="ExternalInput")
    wo = nc.dram_tensor("wo", [HEADS * DH, DMODEL], bf16, kind="ExternalInput")
    yt = nc.dram_tensor("yt", [DMODEL, N], f32, kind="ExternalOutput")

    with tile.TileContext(nc) as tc:
        with (
            tc.tile_pool(name="singles", bufs=1) as singles,
        ):
            x_sb = singles.tile([128, KO, N], bf16)
            wqk_sb = singles.tile([128, KO, HEADS, 128], bf16)
            wv_sb = singles.tile([128, KO, HEADS * DH], bf16)
            wo_sb = singles.tile([128, 2, DMODEL], bf16)
            q_sb = singles.tile([DH, HEADS, N], bf16)
            k_sb = singles.tile([DH, HEADS, N], bf16)
            v_sb = singles.tile([128, JC, HEADS, 2 * DH], bf16)
            att_sb = singles.tile([128, 2, N], bf16)

            nc.sync.dma_start(x_sb[:], xt.ap().rearrange("(ko p) n -> p ko n", p=128))
            nc.sync.dma_start(
                wqk_sb[:], wqk.ap().rearrange("(ko p) (h m) -> p ko h m", p=128, m=128)
            )
            nc.sync.dma_start(wv_sb[:], wv.ap().rearrange("(ko p) v -> p ko v", p=128))
            nc.sync.dma_start(wo_sb[:], wo.ap().rearrange("(c p) n -> p c n", p=128))
            # columns DH.. are all ones: the o-matmul then replicates the
            # softmax denominator across output partitions DH..2DH-1 for
            # free (matmul cost depends only on the moving operand size)
            nc.vector.memset(v_sb[:, :, :, DH:], 1.0)

            loop_cm = tc.For_i(0, repeat, 1) if loop else contextlib.nullcontext()
            with loop_cm:
              for rep in range(1 if loop else repeat):
                # ---- phase A: qkT per head + v (natural layout) ----
                with (
                    tc.tile_pool(name="ps_qk", bufs=2, space="PSUM") as ps_qk,
                    tc.tile_pool(name="ps_v", bufs=4, space="PSUM") as ps_v,
                ):
                    for h in range(HEADS):
                        for ihalf in range(2):
                            pqk = ps_qk.tile([128, I_HALF], f32, tag="qk")
                            for t in range(I_HALF // 512):
                                for ko in range(KO):
                                    nc.tensor.matmul(
                                        pqk[:, t * 512 : (t + 1) * 512],
                                        wqk_sb[:, ko, h, :],
                                        x_sb[:, ko, ihalf * I_HALF + t * 512 :
                                             ihalf * I_HALF + (t + 1) * 512],
                                        start=(ko == 0),
                                        stop=(ko == KO - 1),
                                    )
                            sl = slice(ihalf * I_HALF, (ihalf + 1) * I_HALF)
                            # q copy on DVE, k copy on the (otherwise idle)
                            # scalar engine so neither serializes the PE
                            nc.vector.tensor_copy(q_sb[:, h, sl], pqk[0:DH, :])
                            nc.scalar.activation(
                                k_sb[:, h, sl], pqk[DH:128, :],
                                mybir.ActivationFunctionType.Copy,
                            )
                    for jc in range(JC):
                        pv = ps_v.tile([128, HEADS * DH], f32)
                        for ko in range(KO):
                            nc.tensor.matmul(
                                pv[:],
                                x_sb[:, ko, jc * 128 : (jc + 1) * 128],
                                wv_sb[:, ko, :],
                                start=(ko == 0),
                                stop=(ko == KO - 1),
                            )
                        nc.vector.tensor_copy(
                            v_sb[:, jc, :, 0:DH],
                            pv[:].rearrange("p (h d) -> p h d", d=DH),
                        )

                # ---- phase B: flash attention, software-pipelined so the
                # scores matmul for step t+1 issues before the o-matmul for
                # step t (PE works while ACT computes the exp) ----
                if stages < 2:
                    continue
                with (
                    tc.tile_pool(name="ps_s", bufs=2, space="PSUM") as ps_s,
                    tc.tile_pool(name="ps_o", bufs=2, space="PSUM") as ps_o,
                    tc.tile_pool(name="p_sb", bufs=3) as p_pool,
                    tc.tile_pool(name="rec_sb", bufs=2) as rec_pool,
                ):
                    steps = [
                        (ih, h, jc)
                        for ih in range(2)
                        for h in range(HEADS)
                        for jc in range(JC)
                    ]

                    def s_matmul(step, s_tile):
                        ih, h, jc = step
                        i0 = ih * I_HALF
                        for t in range(I_HALF // 512):
                            nc.tensor.matmul(
                                s_tile[:, t * 512 : (t + 1) * 512],
                                k_sb[:, h, jc * 128 : (jc + 1) * 128],
                                q_sb[:, h, i0 + t * 512 : i0 + (t + 1) * 512],
                                start=True,
                                stop=True,
                            )

                    s_cur = ps_s.tile([128, I_HALF], f32, tag="s")
                    s_matmul(steps[0], s_cur)
                    o = None
                    for t, (ih, h, jc) in enumerate(steps):
                        if jc == 0:
                            o = ps_o.tile([2 * DH, I_HALF], f32)
                        p = p_pool.tile([128, I_HALF], bf16)
                        nc.scalar.activation(
                            p[:], s_cur[:], mybir.ActivationFunctionType.Exp,
                            scale=0.125,
                        )
                        if t + 1 < len(steps):
                            s_next = ps_s.tile([128, I_HALF], f32, tag="s")
                            s_matmul(steps[t + 1], s_next)
                            s_cur = s_next
                        for tt in range(I_HALF // 512):
                            nc.tensor.matmul(
                                o[:, tt * 512 : (tt + 1) * 512],
                                v_sb[:, jc, h, :],
                                p[:, tt * 512 : (tt + 1) * 512],
                                start=(jc == 0),
                                stop=(jc == JC - 1),
                            )
                        if jc == JC - 1:
                            # normalize: o[DH:2DH] holds the denominator
                            # replicated on all 64 partitions, so a single
                            # DVE reciprocal + multiply suffices.
                            i0 = ih * I_HALF
                            rec = rec_pool.tile([DH, I_HALF], f32)
                            nc.vector.reciprocal(rec[:], o[DH:, :])
                            nc.vector.tensor_mul(
                                att_sb[(h % 2) * DH : (h % 2 + 1) * DH, h // 2,
                                       i0 : i0 + I_HALF],
                                o[0:DH, :],
                                rec[:],
                            )

                # ---- phase C: output projection yT = wo.T @ attT ----
                if stages < 4:
                    continue
                # right-side PSUM banks so the next loop iteration's phase A
                # (left-side banks) can start while C's copies/DMA drain
                with (
                    tc.tile_pool(name="ps_y", bufs=2, space="PSUM", side="right") as ps_y,
                    tc.tile_pool(name="y_sb", bufs=2) as y_pool,
                ):
                    for m in range(KO):
                        y_row = y_pool.tile([128, N], f32)
                        for t in range(N // 512):
                            py = ps_y.tile([128, 512], f32)
                            for c in range(2):
                                nc.tensor.matmul(
                                    py[:],
                                    wo_sb[:, c, m * 128 : (m + 1) * 128],
                                    att_sb[:, c, t * 512 : (t + 1) * 512],
                                    start=(c == 0),
                                    stop=(c == 1),
                                )
                            # alternate PSUM->SBUF copies between DVE and ACT
                            if t % 2 == 0:
                                nc.vector.tensor_copy(
                                    y_row[:, t * 512 : (t + 1) * 512], py[:]
                                )
                            else:
                                nc.scalar.activation(
                                    y_row[:, t * 512 : (t + 1) * 512], py[:],
                                    mybir.ActivationFunctionType.Copy,
                                )
                        nc.sync.dma_start(
                            yt.ap().rearrange("(mo p) n -> p mo n", p=128)[:, m, :],
                            y_row[:],
                        )

    if fixup:
        _fixup_drains(nc, mybir)
    return nc


def _per_core_inputs(x, w_qkv, w_out):
    """Slice + transpose the full inputs into the 8 per-core input maps."""
    import ml_dtypes

    bf16 = ml_dtypes.bfloat16
    ins = []
    for c in range(N_CORES):
        b, g = c // 2, c % 2
        xt = np.ascontiguousarray(x[b].T).astype(bf16)          # [512, 2048]
        wq = w_qkv[:, g * 256 : (g + 1) * 256]                  # [512, 256]
        wk = w_qkv[:, 512 + g * 256 : 512 + (g + 1) * 256]
        wv = w_qkv[:, 1024 + g * 256 : 1024 + (g + 1) * 256]
        # per head: [w_q_h | w_k_h] -> [512, 4, 128]
        wqk = np.empty((DMODEL, HEADS, 128), np.float32)
        for h in range(HEADS):
            wqk[:, h, :DH] = wq[:, h * DH : (h + 1) * DH]
            wqk[:, h, DH:] = wk[:, h * DH : (h + 1) * DH]
        wo = w_out[g * 256 : (g + 1) * 256, :]                  # [256, 512]
        ins.append(
            {
                "xt": xt,
                "wqk": np.ascontiguousarray(wqk.reshape(DMODEL, HEADS * 128)).astype(bf16),
                "wv": np.ascontiguousarray(wv).astype(bf16),
                "wo": np.ascontiguousarray(wo).astype(bf16),
            }
        )
    return ins


def run_on_hw(x, w_qkv, w_out, b_out, repeat=1, loop=False):
    from concourse.bass_utils import run_bass_kernel_spmd

    key = ("nc", repeat, loop)
    if key not in _CACHE:
        _CACHE[key] = build_nc(repeat=repeat, loop=loop)
    nc = _CACHE[key]
    ins = _per_core_inputs(
        np.asarray(x, np.float32),
        np.asarray(w_qkv, np.float32),
        np.asarray(w_out, np.float32),
    )
    res = run_bass_kernel_spmd(nc, ins, core_ids=list(range(N_CORES)))
    yts = [res.results[c]["yt"] for c in range(N_CORES)]
    b_out = np.asarray(b_out, np.float32)
    out = np.stack(
        [(yts[2 * b] + yts[2 * b + 1]).T + b_out[None, :] for b in range(4)]
    )
    return out.astype(np.float32)


def kernel(x, w_qkv, w_out, b_out):
    return run_on_hw(x, w_qkv, w_out, b_out, repeat=1, loop=True)

